# revision 1
# baseline (speedup 1.0000x reference)
"""Trainium2 Bass kernel for nn_DecoderBlock (B=4, T=S=1024, DM=1024, H=16, HID=4096).

Sharding: sequence-parallel over T across 8 cores. Core i owns query/token
chunk t in [128*i, 128*(i+1)) for all 4 batches (512 rows, b-major). All
per-token ops (projections, LayerNorm, FFN, residuals) are local; the only
communication is 4 bf16 AllGathers for self/cross attention K^T and V.

Layouts:
  - Activations are feature-major in SBUF: 8 tiles [128 dm, 512 rows] f32r,
    rows = b*128 + t_loc. f32r matmuls at N=512 run at full PE rate with no
    cast passes (weights DMA'd straight into f32r tiles).
  - Attention inner products run in bf16: K^T/Q^T produced feature-major
    [depth, tokens], V token-major [tokens, depth]; scores are computed
    transposed (S^T [kpos, q]) so the AV matmul consumes exp(S^T) directly.
  - Softmax denominator via a ones-vector matmul accumulated next to AV;
    normalization by broadcast-matmul of the reciprocal.
"""
import contextlib
import sys

sys.path.insert(0, "/opt/trn_rl_repo")

import numpy as np

import concourse.bass as bass
import concourse.mybir as mybir
import concourse.tile as tile
from concourse import bacc
from concourse.bass_utils import run_bass_kernel_spmd
from concourse.masks import make_identity

F32 = mybir.dt.float32
F32R = mybir.dt.float32r
BF16 = mybir.dt.bfloat16
AF = mybir.ActivationFunctionType
ALU = mybir.AluOpType

N_CORES = 8
B, T, DM, H, HID = 4, 1024, 1024, 16, 4096
DEPTH = DM // H            # 64
TLOC = T // N_CORES        # 128 tokens per core
ROWS = B * TLOC            # 512 rows per core (b-major)
P = 128
NKT = DM // P              # 8 feature tiles
NHT = HID // P             # 32 hidden tiles
NEG = -1e9
VW = H * (DEPTH + 1)      # V bounce width: 65 cols per head (last is ones)

_CACHE = {}

def _tile(pool, shape, dtype, tag, **kw):
    return pool.tile(shape, dtype, name=tag, tag=tag, **kw)



def _emit(nc, tc, D):
    """Build the whole decoder block inside a TileContext."""
    es = contextlib.ExitStack()
    D["_es"] = es

    def pool(name, **kw):
        return es.enter_context(tc.tile_pool(name=name, **kw))

    scoped = {}

    def pool_open(name, **kw):
        cm = tc.tile_pool(name=name, **kw)
        scoped[name] = cm
        return cm.__enter__()

    def pool_close(name):
        scoped.pop(name).__exit__(None, None, None)

    const = pool("const", bufs=1)
    wpool = pool("wpool", bufs=8)       # streamed weight tiles [128,512] f32r
    spool = pool("spool", bufs=3)        # misc staging
    epool = pool("epool", bufs=3)        # exp(S^T) tiles
    dram = pool("dram", bufs=1, space="DRAM")
    pp = pool("pp", bufs=8, space="PSUM")

    # ---- constants -------------------------------------------------------
    id_r = _tile(const, [P, P], F32, "id_r")
    make_identity(nc, id_r[:])
    id_b = _tile(const, [P, P], BF16, "id_b")
    make_identity(nc, id_b[:])
    ones_col_f = _tile(const, [P, 1], F32, "ones_col_f")
    nc.vector.memset(ones_col_f[:], 1.0)
    ones_col_r = _tile(const, [P, 1], F32R, "ones_col_r")
    nc.vector.tensor_copy(out=ones_col_r[:], in_=ones_col_f[:])
    ones_col_b = _tile(const, [P, 1], BF16, "ones_col_b")
    nc.vector.memset(ones_col_b[:], 1.0)
    ones_row_f = _tile(const, [1, P], F32, "ones_row_f")
    nc.vector.memset(ones_row_f[:], 1.0)
    ones_row_r = _tile(const, [1, P], F32R, "ones_row_r")
    nc.vector.tensor_copy(out=ones_row_r[:], in_=ones_row_f[:])
    ones_row_b = _tile(const, [1, P], BF16, "ones_row_b")
    nc.vector.memset(ones_row_b[:], 1.0)
    ones_sb16 = _tile(const, [P, H], BF16, "ones_sb16")
    nc.vector.memset(ones_sb16[:], 1.0)
    eps_t = _tile(const, [1, 1], F32, "eps_t")
    nc.vector.memset(eps_t[:], 1e-6)

    def vec_tiles(name, n=NKT, scale=None):
        """DRAM [n*128] vector -> n SBUF [128,1] f32 tiles."""
        v = D[name]
        out = []
        for j in range(n):
            t = _tile(const, [P, 1], F32, f"{name}_{j}")
            nc.sync.dma_start(t[:], v[j * P:(j + 1) * P][:, None])
            if scale is not None:
                nc.vector.tensor_scalar_mul(t[:], t[:], scale)
            out.append(t)
        return out

    def row_halves(name):
        """DRAM [1024] -> two [1, 512] f32r tiles (free-dim bias rows)."""
        v = D[name]
        out = []
        for g in range(2):
            t = _tile(const, [1, 512], F32R, f"{name}_row{g}")
            nc.sync.dma_start(t[:], v[g * 512:(g + 1) * 512][None, :].bitcast(F32R))
            out.append(t)
        return out

    bQ1 = vec_tiles("bq1", scale=0.125)
    bK1 = vec_tiles("bk1")
    bV1 = row_halves("bv1")
    bO1 = vec_tiles("bo1")
    bQ2 = vec_tiles("bq2", scale=0.125)
    bK2 = vec_tiles("bk2")
    bV2 = row_halves("bv2")
    bO2 = vec_tiles("bo2")
    bH = vec_tiles("bh", n=NHT)
    bOUT = vec_tiles("bout")
    G1, BE1 = vec_tiles("g1"), vec_tiles("be1")
    G2, BE2 = vec_tiles("g2"), vec_tiles("be2")
    G3, BE3 = vec_tiles("g3"), vec_tiles("be3")

    # ---- entry transposes: [4,128,1024] f32 token-major -> 8 x [128,512] f32r
    def entry_T(src, tagp, tpool):
        outs = [_tile(tpool, [P, ROWS], F32R, f"{tagp}{j}") for j in range(NKT)]
        for b in range(B):
            row = _tile(spool, [P, DM], F32, "entry_row", bufs=1)
            nc.sync.dma_start(row[:], src[b])
            for j in range(NKT):
                ps = _tile(pp, [P, P], F32, "ps")
                nc.tensor.transpose(ps[:], row[:, j * P:(j + 1) * P], id_r[:])
                nc.scalar.activation(outs[j][:, b * P:(b + 1) * P], ps[:], AF.Copy)
        return outs

    p_pre = pool_open("p_pre", bufs=1)
    p_ao = pool_open("p_ao", bufs=1)
    kpool = pool_open("kpool", bufs=1)   # gathered K tiles per b
    vpool = pool_open("vpool", bufs=1)   # gathered V tiles per b
    p_x = pool_open("p_x", bufs=1)
    xT = entry_T(D["xq"], "xT", p_x)

    # ---- mask prep: maskq [4,128,1024] -> maskT [4][2] of [128,512] bf16*(-1e9)
    maskT = []
    for b in range(B):
        row = _tile(spool, [P, T], F32, "mask_row", bufs=1)
        nc.sync.dma_start(row[:], D["maskq"][b])
        mrow = _tile(spool, [P, T], BF16, "mask_scaled", bufs=1)
        nc.vector.tensor_scalar_mul(mrow[:], row[:], NEG)
        gtiles = []
        for g in range(2):
            mt = _tile(const, [P, 512], BF16, f"maskT_{b}_{g}")
            for c in range(4):
                j = 4 * g + c
                ps = _tile(pp, [P, P], BF16, "ps")
                nc.tensor.transpose(ps[:], mrow[:, j * P:(j + 1) * P], id_b[:])
                nc.scalar.activation(mt[:, c * P:(c + 1) * P], ps[:], AF.Copy)
            gtiles.append(mt)
        maskT.append(gtiles)

    # ---- generic projection helpers -------------------------------------
    def load_w_tiles(wname, kt, g):
        """Weight tile [128k, 512 dout] f32r from DRAM [K, N]."""
        t = _tile(wpool, [P, 512], F32R, "w")
        nc.sync.dma_start(t[:], D[wname][kt * P:(kt + 1) * P,
                                         g * 512:(g + 1) * 512].bitcast(F32R))
        return t

    def proj_feature_major(wname, actT, evict):
        """out^T[dout, rows] = w^T @ act^T ; evict(psum, d) per dout tile.

        k-outer loop: 4 dout psums accumulate in parallel while weight
        tiles stream through a small ring (each tile read 4x then freed).
        """
        ng = {"wh": 8}.get(wname, 2)
        nkt = {"wout": NHT}.get(wname, NKT)
        for g in range(ng):
            pss = [_tile(pp, [P, ROWS], F32, "ps") for _ in range(4)]
            for k in range(nkt):
                wt = load_w_tiles(wname, k, g)
                for c in range(4):
                    nc.tensor.matmul(pss[c][:], wt[:, c * P:(c + 1) * P],
                                     actT[k][:], start=(k == 0),
                                     stop=(k == nkt - 1))
            for c in range(4):
                evict(pss[c], 4 * g + c)

    def proj_token_major(wname, actT, brow, bounce):
        """V = act @ w + b, token-major [rows, dout]; DMA into bounce DRAM."""
        for g in range(2):
            pss = [_tile(pp, [P, 512], F32, "ps") for _ in range(4)]
            for k in range(NKT):
                wt = load_w_tiles(wname, k, g)
                for r in range(4):
                    nc.tensor.matmul(pss[r][:], actT[k][:, r * P:(r + 1) * P],
                                     wt[:], start=(k == 0), stop=False)
            for r in range(4):
                nc.tensor.matmul(pss[r][:], ones_row_r[:, :P], brow[g][:],
                                 start=False, stop=True)
                sb = _tile(spool, [P, 512], BF16, "v_evict", bufs=2)
                nc.scalar.activation(sb[:], pss[r][:], AF.Copy)
                dst = bounce[:].rearrange("r (h c) -> r h c", c=DEPTH + 1)[
                    r * P:(r + 1) * P, g * 8:(g + 1) * 8, 0:DEPTH]
                nc.sync.dma_start(
                    dst, sb[:].rearrange("p (h c) -> p h c", c=DEPTH))
        for r in range(4):
            nc.sync.dma_start(
                bounce[:].rearrange("r (h c) -> r h c", c=DEPTH + 1)[
                    r * P:(r + 1) * P, :, DEPTH:DEPTH + 1],
                ones_sb16[:, :, None])

    # ---- K/V projections + AllGathers (issued as early as possible) ------
    def kv_and_ag(actT, wk_name, wv_name, bk, bv, tagp):
        k_in = _tile(dram, [DM, ROWS], BF16, f"{tagp}k_in")
        v_in = _tile(dram, [ROWS, VW], BF16, f"{tagp}v_in")
        k_g = _tile(dram, [N_CORES * DM, ROWS], BF16, f"{tagp}k_g", addr_space="Shared")
        v_g = _tile(dram, [N_CORES * ROWS, VW], BF16, f"{tagp}v_g", addr_space="Shared")

        def evict_k(ps, d):
            sb = _tile(spool, [P, ROWS], BF16, "k_evict", bufs=2)
            nc.scalar.activation(sb[:], ps[:], AF.Identity, bias=bk[d][:])
            nc.sync.dma_start(k_in[d * P:(d + 1) * P, :], sb[:])

        proj_feature_major(wk_name, actT, evict_k)
        nc.gpsimd.collective_compute(
            "AllGather", ALU.bypass,
            replica_groups=[list(range(N_CORES))],
            ins=[k_in[:].opt()], outs=[k_g[:].opt()])
        proj_token_major(wv_name, actT, bv, v_in)
        nc.gpsimd.collective_compute(
            "AllGather", ALU.bypass,
            replica_groups=[list(range(N_CORES))],
            ins=[v_in[:].opt()], outs=[v_g[:].opt()])
        return k_g, v_g

    k1g, v1g = kv_and_ag(xT, "wk1", "wv1", bK1, bV1, "s")

    p_enc = pool_open("p_enc", bufs=1)
    encT = entry_T(D["enc"], "encT", p_enc)
    k2g, v2g = kv_and_ag(encT, "wk2", "wv2", bK2, bV2, "c")
    pool_close("p_enc")

    # ---- Q projection -> bf16 feature-major tiles ------------------------
    def q_proj(wname, actT, bq, tagp, tpool):
        qT = [_tile(tpool, [P, ROWS], BF16, f"{tagp}{j}") for j in range(NKT)]

        def evict_q(ps, d):
            nc.scalar.activation(qT[d][:], ps[:], AF.Identity,
                                 bias=bq[d][:], scale=0.125)
        proj_feature_major(wname, actT, evict_q)
        return qT

    p_q1 = pool_open("p_q1", bufs=1)
    q1T = q_proj("wq1", xT, bQ1, "q1T", p_q1)

    # ---- attention core --------------------------------------------------
    def attention(qT, k_g, v_g, masked, aoT):
        for b in range(B):
            ktiles = []   # [j][p] -> [128,128] bf16 (kpos-tile j, dm-tile p)
            for j in range(N_CORES):
                tj = []
                for p in range(NKT):
                    t = _tile(kpool, [P, P], BF16, f"kt{j}_{p}")
                    nc.sync.dma_start(
                        t[:], k_g[j * DM + p * P: j * DM + (p + 1) * P,
                                  b * P:(b + 1) * P])
                    tj.append(t)
                ktiles.append(tj)
            vtiles = []
            for j in range(N_CORES):
                t = _tile(vpool, [P, VW], BF16, f"vt{j}")
                nc.sync.dma_start(
                    t[:], v_g[j * ROWS + b * P: j * ROWS + (b + 1) * P, :])
                vtiles.append(t)
            for h in range(H):
                hp, ho = h // 2, (h % 2) * DEPTH
                qs = qT[hp][ho:ho + DEPTH, b * P:(b + 1) * P]
                exps = []
                for g in range(2):
                    ps = _tile(pp, [P, 512], F32, "ps")
                    for c in range(4):
                        j = 4 * g + c
                        nc.tensor.matmul(
                            ps[:, c * P:(c + 1) * P],
                            ktiles[j][hp][ho:ho + DEPTH, :], qs,
                            start=True, stop=True)
                    if masked:
                        nc.vector.tensor_add(ps[:], ps[:], maskT[b][g][:])
                    ex = _tile(epool, [P, 512], BF16, "expS")
                    nc.scalar.activation(ex[:], ps[:], AF.Exp)
                    exps.append(ex)
                av = _tile(pp, [DEPTH + 1, P], F32, "ps")
                for g in range(2):
                    for c in range(4):
                        j = 4 * g + c
                        nc.tensor.matmul(
                            av[:], vtiles[j][:, h * (DEPTH + 1):(h + 1) * (DEPTH + 1)],
                            exps[g][:, c * P:(c + 1) * P],
                            start=(j == 0), stop=(j == N_CORES - 1))
                recip = _tile(spool, [1, P], F32, "recip")
                nc.vector.reciprocal(recip[:], av[DEPTH:DEPTH + 1, :])
                bcs = _tile(spool, [DEPTH, P], F32, "bcast_sb")
                nc.gpsimd.partition_broadcast(bcs[:], recip[:])
                nc.vector.tensor_mul(aoT[hp][ho:ho + DEPTH, b * P:(b + 1) * P],
                                     av[0:DEPTH, :], bcs[:])

    aoT = [_tile(p_ao, [P, ROWS], F32R, f"aoT{j}") for j in range(NKT)]
    attention(q1T, k1g, v1g, True, aoT)
    pool_close("p_q1")

    # ---- out-projection + residual + LN ---------------------------------
    def layer_norm(vT, G, BE, out_dtype, tagp, tpool):
        """Feature-major LN over dm (partition axis) via ones-matmuls."""
        s_ps = _tile(pp, [1, ROWS], F32, "ps")
        q_ps = _tile(pp, [1, ROWS], F32, "ps")
        for k in range(NKT):
            nc.tensor.matmul(s_ps[:], ones_col_r[:], vT[k][:],
                             start=(k == 0), stop=(k == NKT - 1))
        for k in range(NKT):
            sq = _tile(spool, [P, ROWS], F32R, "ln_sq", bufs=2)
            nc.vector.tensor_mul(sq[:], vT[k][:], vT[k][:])
            nc.tensor.matmul(q_ps[:], ones_col_r[:], sq[:],
                             start=(k == 0), stop=(k == NKT - 1))
        mean = _tile(spool, [1, ROWS], F32, "ln_mean")
        nc.vector.tensor_scalar_mul(mean[:], s_ps[:], 1.0 / DM)
        ex2 = _tile(spool, [1, ROWS], F32, "ln_ex2")
        nc.vector.tensor_scalar_mul(ex2[:], q_ps[:], 1.0 / DM)
        var = _tile(spool, [1, ROWS], F32, "ln_var")
        nc.vector.scalar_tensor_tensor(var[:], mean[:], -1.0, mean[:],
                                       op0=ALU.mult, op1=ALU.mult)
        nc.vector.tensor_add(var[:], var[:], ex2[:])
        std = _tile(spool, [1, ROWS], F32, "ln_std")
        nc.scalar.activation(std[:], var[:], AF.Sqrt, bias=eps_t[:])
        rstd = _tile(spool, [1, ROWS], F32R, "ln_rstd")
        with nc.allow_low_precision(reason="f32r rstd keeps full f32 bits"):
            nc.vector.reciprocal(rstd[:], std[:])
        nm = _tile(spool, [1, ROWS], F32R, "ln_nm")
        nc.vector.scalar_tensor_tensor(nm[:], mean[:], -1.0, rstd[:],
                                       op0=ALU.mult, op1=ALU.mult)
        r_ps = _tile(pp, [P, ROWS], F32, "ps")
        nc.tensor.matmul(r_ps[:], ones_row_r[:], rstd[:], start=True, stop=True)
        n_ps = _tile(pp, [P, ROWS], F32, "ps")
        nc.tensor.matmul(n_ps[:], ones_row_r[:], nm[:], start=True, stop=True)
        outs = []
        for k in range(NKT):
            tmp = _tile(spool, [P, ROWS], F32, "ln_tmp", bufs=2)
            nc.vector.tensor_mul(tmp[:], vT[k][:], r_ps[:])
            nc.vector.tensor_add(tmp[:], tmp[:], n_ps[:])
            o = _tile(tpool, [P, ROWS], out_dtype, f"{tagp}{k}")
            nc.scalar.activation(o[:], tmp[:], AF.Identity,
                                 bias=BE[k][:], scale=G[k][:])
            outs.append(o)
        return outs

    def out_proj_resid(wname, inT, bo, residT, tagp, tpool):
        vT = []
        def evict(ps, d):
            o = _tile(tpool, [P, ROWS], F32R, f"pre{d}")
            nc.vector.scalar_tensor_tensor(o[:], ps[:], bo[d][:], residT[d][:],
                                           op0=ALU.add, op1=ALU.add)
            vT.append(o)
        proj_feature_major(wname, inT, evict)
        return vT

    v1 = out_proj_resid("wo1", aoT, bO1, xT, "h1pre", p_pre)
    pool_close("p_x")
    p_h1 = pool_open("p_h1", bufs=1)
    h1T = layer_norm(v1, G1, BE1, F32R, "h1T", p_h1)

    # ---- cross attention -------------------------------------------------
    p_q2 = pool_open("p_q2", bufs=1)
    q2T = q_proj("wq2", h1T, bQ2, "q2T", p_q2)
    attention(q2T, k2g, v2g, False, aoT)
    pool_close("p_q2")
    v2 = out_proj_resid("wo2", aoT, bO2, h1T, "h2pre", p_pre)
    pool_close("p_h1")
    pool_close("vpool")
    pool_close("kpool")
    pool_close("p_ao")
    p_h2 = pool_open("p_h2", bufs=1)
    h2T = layer_norm(v2, G2, BE2, F32R, "h2T", p_h2)

    # ---- FFN -------------------------------------------------------------
    p_u = pool_open("p_u", bufs=1)
    uT = [None] * NHT
    def evict_u(ps, d):
        t = _tile(p_u, [P, ROWS], F32R, f"uT{d}")
        nc.scalar.activation(t[:], ps[:], AF.Relu, bias=bH[d][:])
        uT[d] = t
    proj_feature_major("wh", h2T, evict_u)

    v3 = out_proj_resid("wout", uT, bOUT, h2T, "fpre", p_pre)
    pool_close("p_u")
    p_o = pool_open("p_o", bufs=1)
    oT = layer_norm(v3, G3, BE3, F32, "oT", p_o)

    # ---- exit transpose + store -----------------------------------------
    for j in range(NKT):
        for b in range(B):
            ps = _tile(pp, [P, P], F32, "ps")
            nc.tensor.transpose(ps[:], oT[j][:, b * P:(b + 1) * P], id_r[:])
            sb = _tile(spool, [P, P], F32, "out_sb", bufs=2)
            nc.scalar.activation(sb[:], ps[:], AF.Copy)
            nc.sync.dma_start(D["out"][b][:, j * P:(j + 1) * P], sb[:])
    for name in reversed(list(scoped)):
        scoped.pop(name).__exit__(None, None, None)


def _close_rest(scoped):
    for name in reversed(list(scoped)):
        scoped.pop(name).__exit__(None, None, None)


def build():
    if "nc" in _CACHE:
        return _CACHE["nc"]
    nc = bacc.Bacc("TRN2", target_bir_lowering=False, debug=False,
                   enable_asserts=True, num_devices=N_CORES)
    D = {}
    def inp(name, shape):
        D[name] = nc.dram_tensor(name, list(shape), F32, kind="ExternalInput").ap()
    inp("xq", (B, TLOC, DM))
    inp("enc", (B, TLOC, DM))
    inp("maskq", (B, TLOC, T))
    for w in ["wq1", "wk1", "wv1", "wo1", "wq2", "wk2", "wv2", "wo2"]:
        inp(w, (DM, DM))
        inp("b" + w[1:], (DM,))
    inp("wh", (DM, HID))
    inp("bh", (HID,))
    inp("wout", (HID, DM))
    inp("bout", (DM,))
    for i in (1, 2, 3):
        inp(f"g{i}", (DM,))
        inp(f"be{i}", (DM,))
    D["out"] = nc.dram_tensor("out", [B, TLOC, DM], F32,
                              kind="ExternalOutput").ap()
    with tile.TileContext(nc) as tc:
        _emit(nc, tc, D)
        D["_es"].close()
    nc.compile()
    _CACHE["nc"] = nc
    return nc


def _make_in_maps(inputs):
    x = np.ascontiguousarray(inputs["x"], dtype=np.float32)
    enc = np.ascontiguousarray(inputs["enc_out"], dtype=np.float32)
    mask = np.ascontiguousarray(inputs["look_ahead_mask"], dtype=np.float32)
    shared = {}
    for w in ["wq1", "wk1", "wv1", "wo1", "wq2", "wk2", "wv2", "wo2"]:
        shared[w] = np.ascontiguousarray(inputs[w], dtype=np.float32)
        shared["b" + w[1:]] = np.ascontiguousarray(inputs["b" + w[1:]],
                                                   dtype=np.float32)
    shared["wh"] = np.ascontiguousarray(inputs["wh"], dtype=np.float32)
    shared["bh"] = np.ascontiguousarray(inputs["bh"], dtype=np.float32)
    shared["wout"] = np.ascontiguousarray(inputs["wout"], dtype=np.float32)
    shared["bout"] = np.ascontiguousarray(inputs["bout"], dtype=np.float32)
    for i in (1, 2, 3):
        shared[f"g{i}"] = np.ascontiguousarray(inputs[f"g{i}"], dtype=np.float32)
        shared[f"be{i}"] = np.ascontiguousarray(inputs[f"be{i}"], dtype=np.float32)
    in_maps = []
    for i in range(N_CORES):
        sl = slice(i * TLOC, (i + 1) * TLOC)
        m = dict(shared)
        m["xq"] = np.ascontiguousarray(x[:, sl, :])
        m["enc"] = np.ascontiguousarray(enc[:, sl, :])
        m["maskq"] = np.ascontiguousarray(mask[:, 0, sl, :])
        in_maps.append(m)
    return in_maps


def _assemble(res):
    out = np.empty((B, T, DM), dtype=np.float32)
    for i in range(N_CORES):
        out[:, i * TLOC:(i + 1) * TLOC, :] = res.results[i]["out"]
    return out


def kernel(**inputs):
    nc = build()
    in_maps = _make_in_maps(inputs)
    res = run_bass_kernel_spmd(nc, in_maps, core_ids=list(range(N_CORES)))
    return _assemble(res)



# revision 10
# speedup vs baseline: 1.2422x; 1.2422x over previous
"""Trainium2 Bass kernel for nn_DecoderBlock (B=4, T=S=1024, DM=1024, H=16, HID=4096).

Sharding: sequence-parallel over T across 8 cores. Core i owns token chunk
t in [128*i, 128*(i+1)) for all 4 batches (512 rows, b-major). Per-token ops
(projections, LayerNorm, FFN, residuals) are local; the only communication is
4 bf16 AllGathers for self/cross attention K^T and V.

v2 design notes (vs v1 baseline at 1.29 ms):
  - All matmuls in bf16; weights/activations/masks are cast to bf16 on the
    host, halving HBM weight traffic.
  - DMA batching: weights stream as [128, 4096] tiles (2 KB descriptors),
    K gathers load as 8 big per-j tiles, biases load as [1, N] rows and are
    applied as rank-1 matmul terms accumulated into PSUM (no [128,1] tiles,
    no 4 B descriptors). The mask is pre-transposed/scaled on the host and
    loaded with one DMA. dma_start issue is spread across sync/vector/scalar
    queues (v1 serialized ~1050 issues on the sync sequencer).
  - Softmax: denominator via a ones-column folded into V; per-batch batched
    reciprocal [16,128] + sel2 rank-2 matmul broadcast replaces 128 tiny
    reciprocal + partition_broadcast ops.
  - LayerNorm: gamma/beta applied via rank-1 matmuls from [1,1024] rows.
"""
import contextlib
import sys

sys.path.insert(0, "/opt/trn_rl_repo")

import numpy as np
import ml_dtypes

import concourse.bass as bass
import concourse.mybir as mybir
import concourse.tile as tile
from concourse import bacc
from concourse.bass_utils import run_bass_kernel_spmd
from concourse.masks import make_identity

F32 = mybir.dt.float32
BF16 = mybir.dt.bfloat16
AF = mybir.ActivationFunctionType
ALU = mybir.AluOpType
BF = ml_dtypes.bfloat16

N_CORES = 8
B, T, DM, H, HID = 4, 1024, 1024, 16, 4096
DEPTH = DM // H            # 64
TLOC = T // N_CORES        # 128 tokens per core
ROWS = B * TLOC            # 512 rows per core (b-major)
P = 128
NKT = DM // P              # 8 feature tiles
VW = H * (DEPTH + 1)       # 1040: V bounce width, 65 cols/head (last is ones)

_CACHE = {}


def _emit(nc, tc, D):
    es = contextlib.ExitStack()
    D["_es"] = es

    def pool(name, **kw):
        return es.enter_context(tc.tile_pool(name=name, **kw))

    scoped = {}

    def pool_open(name, **kw):
        cm = tc.tile_pool(name=name, **kw)
        scoped[name] = cm
        return cm.__enter__()

    def pool_close(name):
        scoped.pop(name).__exit__(None, None, None)

    def _tile(pl, shape, dtype, tag, **kw):
        return pl.tile(shape, dtype, name=tag, tag=tag, **kw)

    const = pool("const", bufs=1)
    wpool = pool("wpool", bufs=2)        # [128, 4096] bf16 weight tiles
    spool = pool("spool", bufs=2)        # staging
    epool = pool("epool", bufs=4)        # exp(S^T) tiles
    dram = pool("dram", bufs=1, space="DRAM")
    pp = pool("pp", bufs=4, space="PSUM")

    # ---- constants -------------------------------------------------------
    id_r = _tile(const, [P, P], F32, "id_r")
    make_identity(nc, id_r[:])
    id_b = _tile(const, [P, P], BF16, "id_b")
    make_identity(nc, id_b[:])
    ones_row = _tile(const, [1, 512], BF16, "ones_row")
    nc.vector.memset(ones_row[:], 1.0)
    ones_col = _tile(const, [P, 1], BF16, "ones_col")
    nc.vector.memset(ones_col[:], 1.0)
    eps_t = _tile(const, [1, 1], F32, "eps_t")
    nc.vector.memset(eps_t[:], 1e-6)

    def row(name, tag="brow", n=DM, bufs=2):
        t = _tile(spool, [1, n], BF16, tag, bufs=bufs)
        nc.sync.dma_start(t[:], D[name][None, :])
        return t

    # ---- entry transposes: [4,128,1024] bf16 token-major -> 8 x [128,512]
    def entry_T(src, tagp, tpool, stage):
        outs = [_tile(tpool, [P, ROWS], BF16, f"{tagp}{j}") for j in range(NKT)]
        for b in range(B):
            rw = _tile(stage, [P, DM], BF16, "entry_row", bufs=2)
            nc.sync.dma_start(rw[:], src[b])
            for j in range(NKT):
                ps = _tile(pp, [P, P], BF16, "ps")
                nc.tensor.transpose(ps[:], rw[:, j * P:(j + 1) * P], id_b[:])
                nc.vector.tensor_copy(outs[j][:, b * P:(b + 1) * P], ps[:])
        return outs

    # ---- projections -----------------------------------------------------
    def w_tile(wname, r0, c0):
        """[128, 4096] bf16 weight tile: 8 k-tiles x 512 dout columns."""
        t = _tile(wpool, [P, 4096], BF16, "w")
        src = D[wname][r0:r0 + 1024, c0:c0 + 512].rearrange(
            "(k p) c -> p k c", p=P)
        nc.scalar.dma_start(t[:].rearrange("p (k c) -> p k c", c=512), src)
        return t

    def proj_fm(wname, actT, bname, evict, ng=2, nkc=1):
        """out^T[dout, rows] = w^T @ act^T + b; evict(psum, dout_tile)."""
        brow = row(bname, tag="bigrow" if ng > 2 else "brow",
                   n=512 * ng, bufs=1 if ng > 2 else 2)
        for g in range(ng):
            pss = [_tile(pp, [P, ROWS], F32, "ps") for _ in range(4)]
            for kc in range(nkc):
                wt = w_tile(wname, kc * 1024, g * 512)
                for k8 in range(8):
                    for c in range(4):
                        nc.tensor.matmul(
                            pss[c][:], wt[:, k8 * 512 + c * P:k8 * 512 + (c + 1) * P],
                            actT[kc * 8 + k8][:],
                            start=(kc == 0 and k8 == 0), stop=False)
            for c in range(4):
                d = 4 * g + c
                nc.tensor.matmul(pss[c][:], brow[:, d * P:(d + 1) * P],
                                 ones_row[:], start=False, stop=True)
                evict(pss[c], d)

    def proj_tm(wname, actT, bname, v_in, stage):
        """V = act @ w + b token-major; bounce to DRAM with ones column."""
        brow = row(bname)
        for g in range(2):
            wt = w_tile(wname, 0, g * 512)
            for r in range(4):
                ps = _tile(pp, [P, 512], F32, "ps")
                for k8 in range(8):
                    nc.tensor.matmul(ps[:], actT[k8][:, r * P:(r + 1) * P],
                                     wt[:, k8 * 512:(k8 + 1) * 512],
                                     start=(k8 == 0), stop=False)
                nc.tensor.matmul(ps[:], ones_row[:, 0:P],
                                 brow[:, g * 512:(g + 1) * 512],
                                 start=False, stop=True)
                sb = _tile(stage, [P, 520], BF16, "v_evict", bufs=2)
                nc.scalar.activation(
                    sb[:].rearrange("p (h c) -> p h c", c=DEPTH + 1)[:, :, 0:DEPTH],
                    ps[:].rearrange("p (h c) -> p h c", c=DEPTH), AF.Copy)
                nc.vector.memset(
                    sb[:].rearrange("p (h c) -> p h c", c=DEPTH + 1)[:, :, DEPTH:],
                    1.0)
                nc.sync.dma_start(
                    v_in[r * P:(r + 1) * P, g * 520:(g + 1) * 520], sb[:])

    # ---- K/V projections + AllGathers ------------------------------------
    def kv_and_ag(actT, wk_name, wv_name, bk, bv, tagp, stage):
        k_in = _tile(dram, [DM, ROWS], BF16, f"{tagp}k_in")
        v_in = _tile(dram, [ROWS, VW], BF16, f"{tagp}v_in")
        k_g = _tile(dram, [N_CORES * DM, ROWS], BF16, f"{tagp}k_g",
                    addr_space="Shared")
        v_g = _tile(dram, [N_CORES * ROWS, VW], BF16, f"{tagp}v_g",
                    addr_space="Shared")
        kbuf = _tile(stage, [P, 4096], BF16, "kbuf", bufs=1)

        def evict_k(ps, d):
            nc.scalar.activation(kbuf[:, d * 512:(d + 1) * 512], ps[:], AF.Copy)

        proj_fm(wk_name, actT, bk, evict_k)
        nc.sync.dma_start(
            k_in[:].rearrange("(d p) c -> p d c", p=P),
            kbuf[:].rearrange("p (d c) -> p d c", c=512))
        nc.gpsimd.collective_compute(
            "AllGather", ALU.bypass,
            replica_groups=[list(range(N_CORES))],
            ins=[k_in[:].opt()], outs=[k_g[:].opt()])
        proj_tm(wv_name, actT, bv, v_in, stage)
        nc.gpsimd.collective_compute(
            "AllGather", ALU.bypass,
            replica_groups=[list(range(N_CORES))],
            ins=[v_in[:].opt()], outs=[v_g[:].opt()])
        return k_g, v_g

    # ---- phase 1: entry + K/V + AGs as early as possible -----------------
    p_pre = pool_open("p_pre", bufs=1)
    p_ao = pool_open("p_ao", bufs=1)
    aoT = [_tile(p_ao, [P, ROWS], BF16, f"aoT{j}") for j in range(NKT)]
    pa = pool_open("pa", bufs=1, space="PSUM")
    vpool = pool_open("vpool", bufs=1)
    kpool = pool_open("kpool", bufs=1)
    p_x = pool_open("p_x", bufs=1)
    p_stage = pool_open("p_stage", bufs=1)
    xT = entry_T(D["xq"], "xT", p_x, p_stage)
    k1g, v1g = kv_and_ag(xT, "wk1", "wv1", "bk1", "bv1", "s", p_stage)

    p_enc = pool_open("p_enc", bufs=1)
    encT = entry_T(D["enc"], "encT", p_enc, p_stage)
    k2g, v2g = kv_and_ag(encT, "wk2", "wv2", "bk2", "bv2", "c", p_stage)
    pool_close("p_enc")
    pool_close("p_stage")

    # ---- Q projection -> bf16 feature-major tiles ------------------------
    def q_proj(wname, actT, bq, tagp, tpool):
        qT = [_tile(tpool, [P, ROWS], BF16, f"{tagp}{j}") for j in range(NKT)]

        def evict_q(ps, d):
            nc.scalar.activation(qT[d][:], ps[:], AF.Copy, scale=0.125)
        proj_fm(wname, actT, bq, evict_q)
        return qT

    p_q1 = pool_open("p_q1", bufs=1)
    q1T = q_proj("wq1", xT, "bq1", "q1T", p_q1)

    # ---- masks -----------------------------------------------------------
    p_mask = pool_open("p_mask", bufs=1)
    mask_sb = _tile(p_mask, [P, 4096], BF16, "mask_sb")
    nc.sync.dma_start(
        mask_sb[:].rearrange("p (b g c) -> p b g c", g=2, c=512),
        D["maskt"].rearrange("b g p c -> p b g c"))

    # ---- attention core --------------------------------------------------
    def attention(qT, k_g, v_g, mfn):
        ksb = [_tile(kpool, [P, 4096], BF16, f"k{j}") for j in range(N_CORES)]
        for j in range(N_CORES):
            nc.gpsimd.dma_start(
                ksb[j][:].rearrange("p (t c) -> p t c", c=512),
                k_g[j * DM:(j + 1) * DM, :].rearrange("(t p) c -> p t c", p=P))
        for b in range(B):
            vsb = [_tile(vpool, [P, VW], BF16, f"v{j}", bufs=1)
                   for j in range(N_CORES)]
            for j in range(N_CORES):
                nc.sync.dma_start(
                    vsb[j][:], v_g[j * ROWS + b * P:j * ROWS + (b + 1) * P, :])
            dall = [_tile(spool, [1, NKT * P], F32, f"dall{i}", bufs=1)
                    for i in range(2)]
            dall_r = [_tile(spool, [1, NKT * P], BF16, f"dall_r{i}", bufs=1)
                      for i in range(2)]
            avs = [_tile(pa, [DEPTH + 1, 512], F32, f"av{q}") for q in range(4)]
            for h in range(H):
                hp, ho = h // 2, (h % 2) * DEPTH
                qs = qT[hp][ho:ho + DEPTH, b * P:(b + 1) * P]
                exps = []
                for g in range(2):
                    ps = _tile(pp, [P, 512], F32, "ps")
                    for c in range(4):
                        j = 4 * g + c
                        nc.tensor.matmul(
                            ps[:, c * P:(c + 1) * P],
                            ksb[j][ho:ho + DEPTH, hp * 512 + b * P:hp * 512 + (b + 1) * P],
                            qs, start=True, stop=True)
                    nc.vector.tensor_add(ps[:], ps[:], mfn(b, g))
                    ex = _tile(epool, [P, 512], BF16, "expS")
                    nc.scalar.activation(ex[:], ps[:], AF.Exp)
                    exps.append(ex)
                av, hc = avs[h // 4], (h % 4) * P
                for g in range(2):
                    for c in range(4):
                        j = 4 * g + c
                        nc.tensor.matmul(
                            av[:, hc:hc + P],
                            vsb[j][:, h * (DEPTH + 1):(h + 1) * (DEPTH + 1)],
                            exps[g][:, c * P:(c + 1) * P],
                            start=(j == 0), stop=(j == N_CORES - 1))
                nc.vector.tensor_copy(
                    dall[h % 2][:, (h // 2) * P:(h // 2 + 1) * P],
                    av[DEPTH:DEPTH + 1, hc:hc + P])
            with nc.allow_low_precision(reason="bf16 softmax denom"):
                nc.vector.reciprocal(dall_r[0][:], dall[0][:])
                nc.vector.reciprocal(dall_r[1][:], dall[1][:])
            for hp in range(NKT):
                rps = _tile(pp, [P, P], F32, "ps")
                nc.tensor.matmul(rps[0:DEPTH, :], ones_row[:, 0:DEPTH],
                                 dall_r[0][:, hp * P:(hp + 1) * P],
                                 start=True, stop=True)
                nc.tensor.matmul(rps[DEPTH:P, :], ones_row[:, 0:DEPTH],
                                 dall_r[1][:, hp * P:(hp + 1) * P],
                                 start=True, stop=True)
                rsb = _tile(spool, [P, P], BF16, "rsb", bufs=2)
                nc.scalar.activation(rsb[:], rps[:], AF.Copy)
                for hh in range(2):
                    h = 2 * hp + hh
                    ho = hh * DEPTH
                    av, hc = avs[h // 4], (h % 4) * P
                    nc.vector.tensor_mul(
                        aoT[hp][ho:ho + DEPTH, b * P:(b + 1) * P],
                        av[0:DEPTH, hc:hc + P], rsb[ho:ho + DEPTH, :])

    attention(q1T, k1g, v1g, lambda b, g: mask_sb[:, (b * 2 + g) * 512:(b * 2 + g + 1) * 512])
    pool_close("p_mask")
    pool_close("p_q1")

    # ---- out-projection + residual + LN ----------------------------------
    def layer_norm(vT, gname, bename, out_dtype, tagp, tpool):
        G = row(gname, tag="grow", bufs=1)
        BE = row(bename, tag="berow", bufs=1)
        s_ps = _tile(pp, [1, ROWS], F32, "ps")
        q_ps = _tile(pp, [1, ROWS], F32, "ps")
        for k in range(NKT):
            nc.tensor.matmul(s_ps[:], ones_col[:], vT[k][:],
                             start=(k == 0), stop=(k == NKT - 1))
        for k in range(NKT):
            sq = _tile(spool, [P, ROWS], BF16, "ln_sq", bufs=2)
            nc.vector.tensor_mul(sq[:], vT[k][:], vT[k][:])
            nc.tensor.matmul(q_ps[:], ones_col[:], sq[:],
                             start=(k == 0), stop=(k == NKT - 1))
        mean = _tile(spool, [1, ROWS], F32, "ln_mean", bufs=1)
        nc.vector.tensor_scalar_mul(mean[:], s_ps[:], 1.0 / DM)
        ex2 = _tile(spool, [1, ROWS], F32, "ln_ex2", bufs=1)
        nc.vector.tensor_scalar_mul(ex2[:], q_ps[:], 1.0 / DM)
        var = _tile(spool, [1, ROWS], F32, "ln_var", bufs=1)
        nc.vector.scalar_tensor_tensor(var[:], mean[:], -1.0, mean[:],
                                       op0=ALU.mult, op1=ALU.mult)
        nc.vector.tensor_add(var[:], var[:], ex2[:])
        std = _tile(spool, [1, ROWS], F32, "ln_std", bufs=1)
        nc.scalar.activation(std[:], var[:], AF.Sqrt, bias=eps_t[:])
        rstd = _tile(spool, [1, ROWS], F32, "ln_rstd", bufs=1)
        nc.vector.reciprocal(rstd[:], std[:])
        nm = _tile(spool, [1, ROWS], F32, "ln_nm", bufs=1)
        nc.vector.scalar_tensor_tensor(nm[:], mean[:], -1.0, rstd[:],
                                       op0=ALU.mult, op1=ALU.mult)
        rstd_b = _tile(spool, [1, ROWS], BF16, "ln_rstd_b", bufs=1)
        nc.scalar.activation(rstd_b[:], rstd[:], AF.Copy)
        nm_b = _tile(spool, [1, ROWS], BF16, "ln_nm_b", bufs=1)
        nc.scalar.activation(nm_b[:], nm[:], AF.Copy)
        outs = []
        for k in range(NKT):
            r_ps = _tile(pp, [P, ROWS], F32, "ps")
            nc.tensor.matmul(r_ps[:], G[:, k * P:(k + 1) * P], rstd_b[:],
                             start=True, stop=True)
            n_ps = _tile(pp, [P, ROWS], F32, "ps")
            nc.tensor.matmul(n_ps[:], G[:, k * P:(k + 1) * P], nm_b[:],
                             start=True, stop=False)
            nc.tensor.matmul(n_ps[:], BE[:, k * P:(k + 1) * P], ones_row[:],
                             start=False, stop=True)
            tmp = _tile(spool, [P, ROWS], F32, "ln_tmp", bufs=2)
            nc.vector.tensor_mul(tmp[:], vT[k][:], r_ps[:])
            o = _tile(tpool, [P, ROWS], out_dtype, f"{tagp}{k}")
            nc.vector.tensor_add(o[:], tmp[:], n_ps[:])
            outs.append(o)
        return outs

    def out_proj_resid(wname, inT, bo, residT, ng=2, nkc=1):
        vT = []

        def evict(ps, d):
            o = _tile(p_pre, [P, ROWS], BF16, f"pre{d}")
            nc.vector.tensor_add(o[:], ps[:], residT[d][:])
            vT.append(o)
        proj_fm(wname, inT, bo, evict, ng=ng, nkc=nkc)
        return vT

    v1 = out_proj_resid("wo1", aoT, "bo1", xT)
    pool_close("p_x")
    p_h1 = pool_open("p_h1", bufs=1)
    h1T = layer_norm(v1, "g1", "be1", BF16, "h1T", p_h1)

    # ---- cross attention -------------------------------------------------
    p_pad = pool_open("p_pad", bufs=1)
    padrow = row("padt", tag="bigrow", n=B * T, bufs=1)
    padb = []
    for b in range(B):
        pb = _tile(p_pad, [P, 1024], BF16, f"pb{b}")
        for g in range(2):
            ps = _tile(pp, [P, 512], F32, "ps")
            for c in range(4):
                j = 4 * g + c
                nc.tensor.matmul(ps[:, c * P:(c + 1) * P],
                                 padrow[:, b * T + j * P:b * T + (j + 1) * P],
                                 ones_row[:, 0:P], start=True, stop=True)
            nc.scalar.activation(pb[:, g * 512:(g + 1) * 512], ps[:], AF.Copy)
        padb.append(pb)

    p_q2 = pool_open("p_q2", bufs=1)
    q2T = q_proj("wq2", h1T, "bq2", "q2T", p_q2)
    attention(q2T, k2g, v2g, lambda b, g: padb[b][:, g * 512:(g + 1) * 512])
    pool_close("p_q2")
    pool_close("p_pad")
    v2 = out_proj_resid("wo2", aoT, "bo2", h1T)
    pool_close("p_h1")
    pool_close("kpool")
    pool_close("vpool")
    pool_close("pa")
    pool_close("p_ao")
    p_h2 = pool_open("p_h2", bufs=1)
    h2T = layer_norm(v2, "g2", "be2", BF16, "h2T", p_h2)

    # ---- FFN -------------------------------------------------------------
    p_u = pool_open("p_u", bufs=1)
    uT = [None] * 32

    def evict_u(ps, d):
        t = _tile(p_u, [P, ROWS], BF16, f"uT{d}")
        nc.scalar.activation(t[:], ps[:], AF.Relu)
        uT[d] = t
    proj_fm("wh", h2T, "bh", evict_u, ng=8, nkc=1)

    v3 = out_proj_resid("wout", uT, "bout", h2T, ng=2, nkc=4)
    pool_close("p_u")
    p_o = pool_open("p_o", bufs=1)
    oT = layer_norm(v3, "g3", "be3", F32, "oT", p_o)

    # ---- exit transpose + store -----------------------------------------
    for b in range(B):
        ob = _tile(p_o, [P, DM], F32, "ob", bufs=2)
        for half in range(2):
            ps = _tile(pp, [P, 512], F32, "ps")
            for c in range(4):
                j = half * 4 + c
                nc.tensor.transpose(ps[:, c * P:(c + 1) * P],
                                    oT[j][:, b * P:(b + 1) * P], id_r[:])
            nc.scalar.activation(ob[:, half * 512:(half + 1) * 512], ps[:], AF.Copy)
        nc.sync.dma_start(D["out"][b], ob[:])
    for name in reversed(list(scoped)):
        scoped.pop(name).__exit__(None, None, None)


def build():
    if "nc" in _CACHE:
        return _CACHE["nc"]
    nc = bacc.Bacc("TRN2", target_bir_lowering=False, debug=False,
                   enable_asserts=True, num_devices=N_CORES)
    D = {}

    def inp(name, shape, dtype=BF16):
        D[name] = nc.dram_tensor(name, list(shape), dtype,
                                 kind="ExternalInput").ap()
    inp("xq", (B, TLOC, DM))
    inp("enc", (B, TLOC, DM))
    inp("maskt", (B, 2, P, 512))
    inp("padt", (B * T,))
    for w in ["wq1", "wk1", "wv1", "wo1", "wq2", "wk2", "wv2", "wo2"]:
        inp(w, (DM, DM))
        inp("b" + w[1:], (DM,))
    inp("wh", (DM, HID))
    inp("bh", (HID,))
    inp("wout", (HID, DM))
    inp("bout", (DM,))
    for i in (1, 2, 3):
        inp(f"g{i}", (DM,))
        inp(f"be{i}", (DM,))
    D["out"] = nc.dram_tensor("out", [B, TLOC, DM], F32,
                              kind="ExternalOutput").ap()
    with tile.TileContext(nc) as tc:
        _emit(nc, tc, D)
        D["_es"].close()
    nc.compile()
    _CACHE["nc"] = nc
    return nc


def _make_in_maps(inputs):
    x = np.asarray(inputs["x"], dtype=np.float32)
    enc = np.asarray(inputs["enc_out"], dtype=np.float32)
    mask = np.asarray(inputs["look_ahead_mask"], dtype=np.float32)
    pad = np.asarray(inputs["padding_mask"], dtype=np.float32)
    shared = {}
    for w in ["wq1", "wk1", "wv1", "wo1", "wq2", "wk2", "wv2", "wo2",
              "wh", "wout"]:
        shared[w] = np.ascontiguousarray(inputs[w]).astype(BF)
        shared["b" + w[1:]] = np.ascontiguousarray(inputs["b" + w[1:]]).astype(BF)
    for i in (1, 2, 3):
        shared[f"g{i}"] = np.ascontiguousarray(inputs[f"g{i}"]).astype(BF)
        shared[f"be{i}"] = np.ascontiguousarray(inputs[f"be{i}"]).astype(BF)
    shared["padt"] = np.ascontiguousarray(
        (pad[:, 0, 0, :] * -1e9).reshape(B * T)).astype(BF)
    in_maps = []
    for i in range(N_CORES):
        sl = slice(i * TLOC, (i + 1) * TLOC)
        m = dict(shared)
        m["xq"] = np.ascontiguousarray(x[:, sl, :]).astype(BF)
        m["enc"] = np.ascontiguousarray(enc[:, sl, :]).astype(BF)
        msl = mask[:, 0, sl, :]                      # [4, 128(q), 1024(kpos)]
        mt = msl.transpose(0, 2, 1).reshape(B, 2, 4, P, P)
        mt = mt.transpose(0, 1, 3, 2, 4).reshape(B, 2, P, 512)
        m["maskt"] = np.ascontiguousarray(mt * -1e9).astype(BF)
        in_maps.append(m)
    return in_maps


def _assemble(res):
    out = np.empty((B, T, DM), dtype=np.float32)
    for i in range(N_CORES):
        out[:, i * TLOC:(i + 1) * TLOC, :] = res.results[i]["out"]
    return out


def kernel(**inputs):
    nc = build()
    in_maps = _make_in_maps(inputs)
    res = run_bass_kernel_spmd(nc, in_maps, core_ids=list(range(N_CORES)))
    return _assemble(res)
